# revision 55
# baseline (speedup 1.0000x reference)
"""Causal multi-head attention (B=4, N=2048, D=1024, H=16, Dh=64) on 8 TRN2 cores.

Sharding: core c handles batch b=c//2 and head-group g=c%2 (8 of 16 heads).
Megatron-style: Wq/Wkv column-parallel, Wo row-parallel; the per-pair partial
outputs are combined with a bf16 ReduceScatter(add) over core pairs {2b, 2b+1},
then cast back to fp32 on-device.

Everything on-device runs in a transposed layout ([feature, token]) so that no
PE transposes are needed anywhere:
  Qt/Kt = W-stationary matmuls of xT            -> [inner, tok]
  S^T   = Kt-stationary, Qt-moving              -> [key, query]  (2 heads row-packed)
  P^T   = exp(scale*S^T) via ACT, 0/1-masked    -> [key, query]  bf16
  O^T   = V'-stationary ([V | ones]), P^T-moving-> [65, query]   (row 64 = softmax denom)
  out^T = Wo-stationary, O^T-moving             -> [dmodel, tok]
The host pre-transposes and pre-casts x / weights to bf16, so the device does
no fp32->bf16 conversion and input DMA bytes are halved.

Perf structure:
  - warm-up matmuls on scratch SBUF keep the PE HAM clock at 2.4 GHz while the
    initial DMAs stream in;
  - the P@V stage is software-pipelined one key-block behind exp so the PE
    never waits on the ACT engine;
  - softmax denominators are collected at partition offsets {0,32,64,96} (the
    only legal SBUF AP start partitions) and inverted in batched DVE
    reciprocals (4 heads per call) instead of 32 single-partition ones; the
    unnormalized O^T is flushed to SBUF as bf16 so PSUM frees immediately;
  - 1/denom is partition-broadcast with a ones-column f32r matmul on the PE
    (never GpSimd: any in-flight ReduceScatter blocks that queue for its full
    15-75us rendezvous+transfer) and the per-head finishers are woven a few
    attention steps past the reciprocal so the PE stream never waits on DVE;
  - the ReduceScatter output reload is cast to fp32 on ACT, keeping the
    CC-gated copy off the in-order DVE queue;
  - attention-output PSUM tiles rotate through 3 banks so the flush of
    head-pair k overlaps the accumulation of head-pair k+1;
  - projections / output-projection / ReduceScatter of neighbouring spans are
    woven between attention steps to fill PE gaps; the last span does
    per-pair reciprocals and a two-half out-proj / ReduceScatter / store
    pipeline to shorten the kernel tail.
"""

import sys

sys.path.insert(0, "/opt/trn_rl_repo")

import ml_dtypes
import numpy as np

import concourse.bass as bass  # noqa: F401  (kept for parity with framework)
import concourse.mybir as mybir
from concourse import bacc, tile
from concourse.bass_utils import run_bass_kernel_spmd

F32 = mybir.dt.float32
BF16 = mybir.dt.bfloat16
FP8 = mybir.dt.float8e4

B = 4
N = 2048
DM = 1024          # d_model
H = 16
DH = 64
HL = 8             # local heads per core
IL = HL * DH       # 512, local inner dim
SCALE = DH ** -0.5
SPAN = 512         # query-span / matmul moving size
NSP = N // SPAN    # 4
NKB = N // 128     # 16 key/token blocks
NCORES = 8
NWARM = 20         # PE warm-up matmuls covering the initial DMA window
INTERLEAVE = True


def build_program(for_sim=False):
    nc = bacc.Bacc("TRN2", target_bir_lowering=False, debug=False,
                   num_devices=1 if for_sim else NCORES)

    xT_d = nc.dram_tensor("xT", [DM, N], BF16, kind="ExternalInput").ap()
    wq_d = nc.dram_tensor("wq", [DM, IL], BF16, kind="ExternalInput").ap()
    wk_d = nc.dram_tensor("wk", [DM, IL], BF16, kind="ExternalInput").ap()
    wv_d = nc.dram_tensor("wv", [DM, IL], BF16, kind="ExternalInput").ap()
    wo_d = nc.dram_tensor("wo", [IL, DM], BF16, kind="ExternalInput").ap()
    bias_d = nc.dram_tensor("bias", [DM, 1], F32, kind="ExternalInput").ap()
    out_d = nc.dram_tensor("out", [DM // 2, N], F32, kind="ExternalOutput").ap()

    with tile.TileContext(nc) as tc:
        with (
            tc.tile_pool(name="weights", bufs=1) as wpool,
            tc.tile_pool(name="acts", bufs=1) as apool,
            tc.tile_pool(name="work", bufs=3) as work,
            tc.tile_pool(name="psum", bufs=1, space="PSUM") as psum,
            tc.tile_pool(name="dram", bufs=1, space="DRAM") as dram,
        ):
            # ---------- stage 0: PE warm-up + loads (no casts needed) ----
            warm_sb = wpool.tile([128, SPAN], BF16, name="warm_sb")
            nc.vector.memset(warm_sb[:], 0.0)
            ones_f = wpool.tile([1, 128], F32, name="ones_f")
            nc.vector.memset(ones_f[:], 1.0)
            ones_sb = wpool.tile([1, 128], F32R, name="ones_sb")
            nc.vector.tensor_copy(ones_sb[:], ones_f[:])
            for _ in range(NWARM):
                wp = psum.tile([128, SPAN], F32, tag="projrb", bufs=1,
                               name="warm_ps")
                nc.tensor.matmul(wp[:], warm_sb[:, 0:128], warm_sb[:],
                                 start=True, stop=True)

            xh = [apool.tile([128, N], BF16, name=f"xh{pb}", tag=f"xh{pb}")
                  for pb in range(DM // 128)]

            def load_x_span(sp):
                for pb in range(DM // 128):
                    nc.sync.dma_start(
                        xh[pb][:, sp * SPAN:(sp + 1) * SPAN],
                        xT_d[pb * 128:(pb + 1) * 128,
                             sp * SPAN:(sp + 1) * SPAN])

            def load_w(src, n_pb, ncols, nm):
                tiles = []
                for pb in range(n_pb):
                    t = wpool.tile([128, ncols], BF16, name=f"{nm}{pb}",
                                   tag=f"{nm}{pb}")
                    nc.sync.dma_start(t[:], src[pb * 128:(pb + 1) * 128, :])
                    tiles.append(t)
                return tiles

            load_x_span(0)                      # span-0 x first: unblocks PE
            wqh = load_w(wq_d, DM // 128, IL, "wqh")
            wkh = load_w(wk_d, DM // 128, IL, "wkh")
            wvh = load_w(wv_d, DM // 128, IL, "wvh")
            for sp in range(1, NSP):            # prefetch the rest of x
                load_x_span(sp)
            woh = load_w(wo_d, IL // 128, DM, "woh")

            bias_sb = wpool.tile([128, DM // 128], F32, name="bias_sb")
            for mb in range(DM // 128):
                nc.sync.dma_start(bias_sb[:, mb:mb + 1],
                                  bias_d[mb * 128:(mb + 1) * 128, :])

            # 0/1 lower-triangle mask (keep query >= key within a diag block)
            tri_f = work.tile([128, 128], F32, tag="tri_f", bufs=1)
            nc.gpsimd.memset(tri_f[:], 1.0)
            nc.gpsimd.affine_select(
                out=tri_f[:], in_=tri_f[:],
                compare_op=mybir.AluOpType.is_ge,
                fill=0.0, base=0, channel_multiplier=-1,
                pattern=[[1, 128]],
            )
            tri01 = wpool.tile([128, 1, 128], BF16, name="tri01")
            nc.vector.tensor_copy(tri01[:, 0, :], tri_f[:])

            # per-span activation tiles
            qth = [[apool.tile([128, SPAN], BF16, name=f"qt{pb}_{sp}",
                               tag=f"qt{pb}_{sp}")
                    for sp in range(NSP)] for pb in range(IL // 128)]
            kth = [[apool.tile([128, SPAN], BF16, name=f"kt{pb}_{sp}",
                               tag=f"kt{pb}_{sp}")
                    for sp in range(NSP)] for pb in range(IL // 128)]
            vth = [apool.tile([128, HL, DH + 1], BF16, name=f"vt{tb}",
                              tag=f"vt{tb}") for tb in range(NKB)]
            oth = [[apool.tile([128, SPAN], BF16, name=f"ot{pb}_{sp}",
                               tag=f"ot{pb}_{sp}")
                    for sp in range(NSP)] for pb in range(IL // 128)]
            # last span's partials are split into two query-halves so its
            # out-proj / ReduceScatter / store pipeline has a shorter tail
            parts = [dram.tile([DM, SPAN], F32, name=f"part{sp}")
                     for sp in range(NSP - 1)]
            parts_last = [dram.tile([DM, SPAN // 2], F32, name=f"plast{h}")
                          for h in range(2)]

            def qk_group(wt, dst, pb, sp):
                def go():
                    pp = psum.tile([128, SPAN], F32, tag="projrb",
                                   bufs=1, name="pp")
                    for kk in range(DM // 128):
                        nc.tensor.matmul(
                            pp[:],
                            wt[kk][:, pb * 128:(pb + 1) * 128],
                            xh[kk][:, sp * SPAN:(sp + 1) * SPAN],
                            start=(kk == 0), stop=(kk == DM // 128 - 1),
                        )
                    # copy-out on ACT: frees the PSUM slot without queuing
                    # behind DVE's span-boundary normalization bursts
                    nc.scalar.copy(dst[pb][sp][:], pp[:])
                return go

            def v_group(tb):
                def go():
                    pp = psum.tile([128, IL], F32, tag="projrb", bufs=1,
                                   name="ppv")
                    for kk in range(DM // 128):
                        nc.tensor.matmul(
                            pp[:], xh[kk][:, tb * 128:(tb + 1) * 128],
                            wvh[kk][:],
                            start=(kk == 0), stop=(kk == DM // 128 - 1),
                        )
                    nc.scalar.copy(
                        vth[tb][:, :, 0:DH],
                        pp[:].rearrange("p (h d) -> p h d", h=HL))
                    nc.vector.memset(vth[tb][:, :, DH:DH + 1], 1.0)
                return go

            def wo_group(mb, sp, dst=None, c0=0, c1=SPAN):
                w = c1 - c0

                def go():
                    pw = psum.tile([128, SPAN], F32, tag="projrb", bufs=1,
                                   name="pw")
                    for ib in range(IL // 128):
                        nc.tensor.matmul(
                            pw[:, 0:w],
                            woh[ib][:, mb * 128:(mb + 1) * 128],
                            oth[ib][sp][:, c0:c1],
                            start=(ib == 0), stop=(ib == IL // 128 - 1),
                        )
                    po = work.tile([128, SPAN], F32, tag="po", bufs=4,
                                   name="po")
                    nc.vector.tensor_scalar(
                        out=po[:, 0:w], in0=pw[:, 0:w],
                        scalar1=bias_sb[:, mb:mb + 1], scalar2=None,
                        op0=mybir.AluOpType.add,
                    )
                    nc.sync.dma_start(
                        (parts[sp] if dst is None else dst)
                        [mb * 128:(mb + 1) * 128, :], po[:, 0:w])
                return go

            def rs_store(src_parts, sp, c0, w):
                """fp32 ReduceScatter + one direct DRAM->DRAM store.

                No SBUF reload and no cast: nothing downstream of the
                collective ever occupies a compute-engine queue, so a slow
                rendezvous can only delay its own store."""
                if for_sim:
                    src = src_parts
                else:
                    rs = dram.tile([DM // 2, w], F32, name=f"rs{sp}_{c0}")
                    nc.gpsimd.collective_compute(
                        "ReduceScatter", mybir.AluOpType.add,
                        replica_groups=[[0, 1], [2, 3], [4, 5], [6, 7]],
                        ins=[src_parts.opt()], outs=[rs.opt()],
                    )
                    src = rs
                nc.sync.dma_start(
                    out_d[:, sp * SPAN + c0:sp * SPAN + c0 + w],
                    src[0:DM // 2, :])

            def rs_span(sp):
                rs_store(parts[sp], sp, 0, SPAN)

            def proj_tasks(sp, load=True):
                tasks = []
                for wt, dst in ((wqh, qth), (wkh, kth)):
                    for pb in range(IL // 128):
                        tasks.append(qk_group(wt, dst, pb, sp))
                for tb in range(4 * sp, 4 * sp + 4):
                    tasks.append(v_group(tb))
                return tasks

            def attn_s_exp(hp, qs, kb):
                """S^T matmuls + exp for key-block kb; returns the P^T tile."""
                off = kb * 128 - qs * SPAN   # <0 for off-diag
                lo = max(off, 0)             # first causal query
                sg = psum.tile([128, 2, SPAN], F32, tag="sT",
                               bufs=2, name="sg")
                for i in range(2):
                    nc.tensor.matmul(
                        sg[:, i, lo:SPAN],
                        kth[hp][kb // 4][64 * i:64 * i + 64,
                                         (kb % 4) * 128:
                                         (kb % 4) * 128 + 128],
                        qth[hp][qs][64 * i:64 * i + 64, lo:SPAN],
                        start=True, stop=True,
                    )
                pt = work.tile([128, 2, SPAN], BF16, tag="pT",
                               bufs=4, name="pt")
                nc.scalar.activation(
                    pt[:, :, lo:SPAN], sg[:, :, lo:SPAN],
                    mybir.ActivationFunctionType.Exp, scale=SCALE)
                if off >= 0:
                    # zero the strictly-upper triangle of the diagonal
                    # 128x128 block for both heads at once
                    nc.vector.tensor_tensor(
                        out=pt[:, :, lo:lo + 128],
                        in0=pt[:, :, lo:lo + 128],
                        in1=tri01[:].broadcast_to([128, 2, 128]),
                        op=mybir.AluOpType.mult,
                    )
                return pt, lo

            def attn_pv(hp, qs, kb, pt, lo, o_ps, nkb):
                for i in range(2):
                    nc.tensor.matmul(
                        o_ps[i][:, lo:SPAN],
                        vth[kb][:, 2 * hp + i, :],
                        pt[:, i, lo:SPAN],
                        start=(kb == 0), stop=(kb == nkb - 1),
                    )

            def attn_flush(hp, qs, i, o_ps, den4):
                """Copy unnormalized O^T (bf16) + its denom row out of PSUM.

                Copies run on ACT so the PSUM bank's release never queues
                behind DVE's reciprocal bursts. SBUF APs may only start at
                partition 0/32/64/96, so denominators are collected at those
                four offsets, 4 heads per collector tile."""
                h = 2 * hp + i
                t, k = h // 4, h % 4
                nc.vector.tensor_copy(den4[t][32 * k:32 * k + 1, :],
                                      o_ps[i][DH:DH + 1, :])
                nc.vector.tensor_copy(
                    oth[hp][qs][64 * i:64 * i + 64, :], o_ps[i][0:DH, :])

            def emit_half_norm(qs, den4, t, pairs):
                """Reciprocal one den4 collector (or a 33-partition slice of
                it), round it to f32r, and return per-head normalization
                finishers.

                The finisher broadcasts 1/denom to all partitions with a
                ones-column f32r matmul on the PE that reads the rounded
                collector DIRECTLY (stationary and moving share their base
                partition), then scales O^T in place on DVE. Nothing in the
                chain touches GpSimd, whose queue is blocked for the whole
                rendezvous+transfer of any in-flight ReduceScatter (measured
                15-75us), and the PE-side matmul only depends on the
                reciprocal, whose latency is hidden by deferring the
                finishers a few attention steps."""
                if len(pairs) == 2:
                    sl = slice(0, 97)
                else:
                    sl = slice(0, 33) if pairs[0] % 2 == 0 else slice(64, 97)
                nc.vector.reciprocal(den4[t][sl, :], den4[t][sl, :])
                d0s = {}
                for hp in pairs:
                    for i in range(2):
                        h = 2 * hp + i
                        k = h % 4
                        d0 = work.tile([1, SPAN], F32R, tag="d0r",
                                       bufs=6, name="d0r")
                        nc.vector.tensor_copy(
                            d0[:], den4[t][32 * k:32 * k + 1, :])
                        d0s[h] = d0

                def finisher(hp, i):
                    d0 = d0s[2 * hp + i]

                    def go():
                        rbf = psum.tile([128, SPAN], F32, tag="oT",
                                        bufs=3, name="rbf")
                        nc.tensor.matmul(rbf[:], ones_sb[:], d0[:],
                                         start=True, stop=True)
                        nc.vector.tensor_tensor(
                            out=oth[hp][qs][64 * i:64 * i + 64, :],
                            in0=oth[hp][qs][64 * i:64 * i + 64, :],
                            in1=rbf[64 * i:64 * i + 64, :],
                            op=mybir.AluOpType.mult,
                        )
                    return go
                return [finisher(hp, i) for hp in pairs for i in range(2)]

            # prologue: projections for span 0 (x span 0 already loading)
            for t in proj_tasks(0, load=False):
                t()

            normq = []       # deferred normalization finishers (cross-span)
            for sp in range(NSP):
                qs = sp
                nkb = 4 * qs + 4
                # independent PE work to weave into attention stalls:
                # next span's projections + previous span's out-proj
                pending = []
                if sp + 1 < NSP:
                    pending += [("proj", t) for t in proj_tasks(sp + 1)]
                if sp >= 1:
                    pending += [("wo", wo_group(mb, sp - 1))
                                for mb in range(DM // 128)]
                    # nothing but collectives lives on the GpSimd queue, so
                    # rs(sp-1) can ride right behind its wo chains
                    pending += [("rs", lambda sp=sp: rs_span(sp - 1))]
                nsteps = nkb * (HL // 2)
                stride = max(1, nsteps // max(1, len(pending)))
                step = 0
                den4 = [work.tile([97, SPAN], F32, tag=f"den4_{t}",
                                  bufs=2, name=f"den4_{t}")
                        for t in range(2)]
                for t in range(2):
                    nc.vector.memset(den4[t][:], 1.0)
                for hp in range(HL // 2):
                    o_ps = [psum.tile([DH + 1, SPAN], F32, tag="oT",
                                      bufs=3, name=f"o_ps{i}")
                            for i in range(2)]
                    prev = None      # (kb, pt, lo) of the un-issued P@V
                    for kb in range(nkb):
                        pt, lo = attn_s_exp(hp, qs, kb)
                        if prev is not None:
                            attn_pv(hp, qs, prev[0], prev[1], prev[2],
                                    o_ps, nkb)
                        prev = (kb, pt, lo)
                        step += 1
                        if normq and kb >= min(5, nkb - 2):
                            normq.pop(0)()
                        if INTERLEAVE and step % stride == 0 and pending:
                            if pending[0][0] != "proj":
                                # wo/rs read O^T: every pending finisher
                                # (in-place normalization) must come first
                                while normq:
                                    normq.pop(0)()
                            pending.pop(0)[1]()
                    attn_pv(hp, qs, prev[0], prev[1], prev[2], o_ps, nkb)
                    for i in range(2):
                        attn_flush(hp, qs, i, o_ps, den4)
                    # per-pair reciprocal right after each pair's flush
                    # (disjoint 33-partition slices of the den4 collectors),
                    # so the deferred finishers get a whole head-pair of
                    # slack and never stall the PE stream
                    fins = emit_half_norm(qs, den4, hp // 2, [hp])
                    if sp == NSP - 1 and hp == 3:
                        for f in fins:
                            f()
                    else:
                        normq += fins
                while pending:
                    if pending[0][0] != "proj":
                        while normq:
                            normq.pop(0)()
                    pending.pop(0)[1]()
            # epilogue: rs(2), then the last span's out-projection in two
            # query-halves so the second half's matmuls overlap the first
            # half's ReduceScatter + store
            for f in normq:
                f()
            for h in range(2):
                for mb in range(DM // 128):
                    wo_group(mb, NSP - 1, dst=parts_last[h],
                             c0=h * (SPAN // 2), c1=(h + 1) * (SPAN // 2))()
                rs_store(parts_last[h], NSP - 1, h * (SPAN // 2), SPAN // 2)

    nc.compile()
    return nc


_program_cache = None


def make_in_maps(inputs):
    bf16 = ml_dtypes.bfloat16
    x = np.asarray(inputs["x"], dtype=np.float32)
    Wq = np.asarray(inputs["Wq"], dtype=np.float32).astype(bf16)
    Wkv = np.asarray(inputs["Wkv"], dtype=np.float32).astype(bf16)
    Wo = np.asarray(inputs["Wo"], dtype=np.float32).astype(bf16)
    bo = np.asarray(inputs["bo"], dtype=np.float32)
    in_maps = []
    for c in range(NCORES):
        b, g = c // 2, c % 2
        in_maps.append({
            "xT": np.ascontiguousarray(x[b].T).astype(bf16),
            "wq": np.ascontiguousarray(Wq[:, g * IL:(g + 1) * IL]),
            "wk": np.ascontiguousarray(Wkv[:, g * IL:(g + 1) * IL]),
            "wv": np.ascontiguousarray(Wkv[:, DM + g * IL:DM + (g + 1) * IL]),
            "wo": np.ascontiguousarray(Wo[g * IL:(g + 1) * IL, :]),
            "bias": (bo if g == 0 else np.zeros_like(bo)).reshape(DM, 1),
        })
    return in_maps


def kernel(x, Wq, Wkv, Wo, bo):
    global _program_cache
    if _program_cache is None:
        _program_cache = build_program()
    nc = _program_cache

    in_maps = make_in_maps(dict(x=x, Wq=Wq, Wkv=Wkv, Wo=Wo, bo=bo))
    res = run_bass_kernel_spmd(nc, in_maps, list(range(NCORES)))

    out = np.empty((B, N, DM), dtype=np.float32)
    for b in range(B):
        top = res.results[2 * b]["out"]       # dmodel rows 0:512
        bot = res.results[2 * b + 1]["out"]   # dmodel rows 512:1024
        out[b] = np.concatenate([top, bot], axis=0).T
    return out


# revision 56
# speedup vs baseline: 1.1271x; 1.1271x over previous
"""Causal multi-head attention (B=4, N=2048, D=1024, H=16, Dh=64) on 8 TRN2 cores.

Sharding: core c handles batch b=c//2 and head-group g=c%2 (8 of 16 heads).
Megatron-style: Wq/Wkv column-parallel, Wo row-parallel; the per-pair partial
outputs are combined with a bf16 ReduceScatter(add) over core pairs {2b, 2b+1},
then cast back to fp32 on-device.

Everything on-device runs in a transposed layout ([feature, token]) so that no
PE transposes are needed anywhere:
  Qt/Kt = W-stationary matmuls of xT            -> [inner, tok]
  S^T   = Kt-stationary, Qt-moving              -> [key, query]  (2 heads row-packed)
  P^T   = exp(scale*S^T) via ACT, 0/1-masked    -> [key, query]  bf16
  O^T   = V'-stationary ([V | ones]), P^T-moving-> [65, query]   (row 64 = softmax denom)
  out^T = Wo-stationary, O^T-moving             -> [dmodel, tok]
The host pre-transposes and pre-casts x / weights to bf16, so the device does
no fp32->bf16 conversion and input DMA bytes are halved.

Perf structure:
  - warm-up matmuls on scratch SBUF keep the PE HAM clock at 2.4 GHz while the
    initial DMAs stream in;
  - the P@V stage is software-pipelined one key-block behind exp so the PE
    never waits on the ACT engine;
  - softmax denominators are collected at partition offsets {0,32,64,96} (the
    only legal SBUF AP start partitions) and inverted in batched DVE
    reciprocals (4 heads per call) instead of 32 single-partition ones; the
    unnormalized O^T is flushed to SBUF as bf16 so PSUM frees immediately;
  - 1/denom is partition-broadcast with a ones-column f32r matmul on the PE
    (never GpSimd: any in-flight ReduceScatter blocks that queue for its full
    15-75us rendezvous+transfer) and the per-head finishers are woven a few
    attention steps past the reciprocal so the PE stream never waits on DVE;
  - the ReduceScatter output reload is cast to fp32 on ACT, keeping the
    CC-gated copy off the in-order DVE queue;
  - attention-output PSUM tiles rotate through 3 banks so the flush of
    head-pair k overlaps the accumulation of head-pair k+1;
  - projections / output-projection / ReduceScatter of neighbouring spans are
    woven between attention steps to fill PE gaps; the last span does
    per-pair reciprocals and a two-half out-proj / ReduceScatter / store
    pipeline to shorten the kernel tail.
"""

import sys

sys.path.insert(0, "/opt/trn_rl_repo")

import ml_dtypes
import numpy as np

import concourse.bass as bass  # noqa: F401  (kept for parity with framework)
import concourse.mybir as mybir
from concourse import bacc, tile
from concourse.bass_utils import run_bass_kernel_spmd

F32 = mybir.dt.float32
BF16 = mybir.dt.bfloat16
FP8 = mybir.dt.float8e4

B = 4
N = 2048
DM = 1024          # d_model
H = 16
DH = 64
HL = 8             # local heads per core
IL = HL * DH       # 512, local inner dim
SCALE = DH ** -0.5
SPAN = 512         # query-span / matmul moving size
NSP = N // SPAN    # 4
NKB = N // 128     # 16 key/token blocks
NCORES = 8
NWARM = 20         # PE warm-up matmuls covering the initial DMA window
INTERLEAVE = True


def build_program(for_sim=False):
    nc = bacc.Bacc("TRN2", target_bir_lowering=False, debug=False,
                   num_devices=1 if for_sim else NCORES)

    xT_d = nc.dram_tensor("xT", [DM, N], BF16, kind="ExternalInput").ap()
    wq_d = nc.dram_tensor("wq", [DM, IL], BF16, kind="ExternalInput").ap()
    wk_d = nc.dram_tensor("wk", [DM, IL], BF16, kind="ExternalInput").ap()
    wv_d = nc.dram_tensor("wv", [DM, IL], BF16, kind="ExternalInput").ap()
    wo_d = nc.dram_tensor("wo", [IL, DM], BF16, kind="ExternalInput").ap()
    bias_d = nc.dram_tensor("bias", [DM, 1], F32, kind="ExternalInput").ap()
    out_d = nc.dram_tensor("out", [DM // 2, N], F32, kind="ExternalOutput").ap()

    with tile.TileContext(nc) as tc:
        with (
            tc.tile_pool(name="weights", bufs=1) as wpool,
            tc.tile_pool(name="acts", bufs=1) as apool,
            tc.tile_pool(name="work", bufs=3) as work,
            tc.tile_pool(name="psum", bufs=1, space="PSUM") as psum,
            tc.tile_pool(name="dram", bufs=1, space="DRAM") as dram,
        ):
            # ---------- stage 0: PE warm-up + loads (no casts needed) ----
            warm_sb = wpool.tile([128, SPAN], BF16, name="warm_sb")
            nc.vector.memset(warm_sb[:], 0.0)
            ones_f = wpool.tile([1, 128], F32, name="ones_f")
            nc.vector.memset(ones_f[:], 1.0)
            ones_sb = wpool.tile([1, 128], F32R, name="ones_sb")
            nc.vector.tensor_copy(ones_sb[:], ones_f[:])
            for _ in range(NWARM):
                wp = psum.tile([128, SPAN], F32, tag="projrb", bufs=2,
                               name="warm_ps")
                nc.tensor.matmul(wp[:], warm_sb[:, 0:128], warm_sb[:],
                                 start=True, stop=True)

            xh = [apool.tile([128, N], BF16, name=f"xh{pb}", tag=f"xh{pb}")
                  for pb in range(DM // 128)]

            def load_x_span(sp):
                for pb in range(DM // 128):
                    nc.sync.dma_start(
                        xh[pb][:, sp * SPAN:(sp + 1) * SPAN],
                        xT_d[pb * 128:(pb + 1) * 128,
                             sp * SPAN:(sp + 1) * SPAN])

            def load_w(src, n_pb, ncols, nm):
                tiles = []
                for pb in range(n_pb):
                    t = wpool.tile([128, ncols], BF16, name=f"{nm}{pb}",
                                   tag=f"{nm}{pb}")
                    nc.sync.dma_start(t[:], src[pb * 128:(pb + 1) * 128, :])
                    tiles.append(t)
                return tiles

            load_x_span(0)                      # span-0 x first: unblocks PE
            wqh = load_w(wq_d, DM // 128, IL, "wqh")
            wkh = load_w(wk_d, DM // 128, IL, "wkh")
            wvh = load_w(wv_d, DM // 128, IL, "wvh")
            for sp in range(1, NSP):            # prefetch the rest of x
                load_x_span(sp)
            woh = load_w(wo_d, IL // 128, DM, "woh")

            bias_sb = wpool.tile([128, DM // 128], F32, name="bias_sb")
            for mb in range(DM // 128):
                nc.sync.dma_start(bias_sb[:, mb:mb + 1],
                                  bias_d[mb * 128:(mb + 1) * 128, :])

            # 0/1 lower-triangle mask (keep query >= key within a diag block)
            tri_f = work.tile([128, 128], F32, tag="tri_f", bufs=1)
            nc.gpsimd.memset(tri_f[:], 1.0)
            nc.gpsimd.affine_select(
                out=tri_f[:], in_=tri_f[:],
                compare_op=mybir.AluOpType.is_ge,
                fill=0.0, base=0, channel_multiplier=-1,
                pattern=[[1, 128]],
            )
            tri01 = wpool.tile([128, 1, 128], BF16, name="tri01")
            nc.vector.tensor_copy(tri01[:, 0, :], tri_f[:])

            # per-span activation tiles
            qth = [[apool.tile([128, SPAN], BF16, name=f"qt{pb}_{sp}",
                               tag=f"qt{pb}_{sp}")
                    for sp in range(NSP)] for pb in range(IL // 128)]
            kth = [[apool.tile([128, SPAN], BF16, name=f"kt{pb}_{sp}",
                               tag=f"kt{pb}_{sp}")
                    for sp in range(NSP)] for pb in range(IL // 128)]
            vth = [apool.tile([128, HL, DH + 1], BF16, name=f"vt{tb}",
                              tag=f"vt{tb}") for tb in range(NKB)]
            oth = [[apool.tile([128, SPAN], BF16, name=f"ot{pb}_{sp}",
                               tag=f"ot{pb}_{sp}")
                    for sp in range(NSP)] for pb in range(IL // 128)]
            # last span's partials are split into two query-halves so its
            # out-proj / ReduceScatter / store pipeline has a shorter tail
            parts = [dram.tile([DM, SPAN], F32, name=f"part{sp}")
                     for sp in range(NSP - 1)]
            parts_last = [dram.tile([DM, SPAN // 2], F32, name=f"plast{h}")
                          for h in range(2)]

            def qk_group(wt, dst, pb, sp):
                def go():
                    pp = psum.tile([128, SPAN], F32, tag="projrb",
                                   bufs=2, name="pp")
                    for kk in range(DM // 128):
                        nc.tensor.matmul(
                            pp[:],
                            wt[kk][:, pb * 128:(pb + 1) * 128],
                            xh[kk][:, sp * SPAN:(sp + 1) * SPAN],
                            start=(kk == 0), stop=(kk == DM // 128 - 1),
                        )
                    # copy-out on ACT: frees the PSUM slot without queuing
                    # behind DVE's span-boundary normalization bursts
                    nc.scalar.copy(dst[pb][sp][:], pp[:])
                return go

            def v_group(tb):
                def go():
                    pp = psum.tile([128, IL], F32, tag="projrb", bufs=2,
                                   name="ppv")
                    for kk in range(DM // 128):
                        nc.tensor.matmul(
                            pp[:], xh[kk][:, tb * 128:(tb + 1) * 128],
                            wvh[kk][:],
                            start=(kk == 0), stop=(kk == DM // 128 - 1),
                        )
                    nc.scalar.copy(
                        vth[tb][:, :, 0:DH],
                        pp[:].rearrange("p (h d) -> p h d", h=HL))
                    nc.vector.memset(vth[tb][:, :, DH:DH + 1], 1.0)
                return go

            def wo_group(mb, sp, dst=None, c0=0, c1=SPAN):
                w = c1 - c0

                def go():
                    pw = psum.tile([128, SPAN], F32, tag="projrb", bufs=2,
                                   name="pw")
                    for ib in range(IL // 128):
                        nc.tensor.matmul(
                            pw[:, 0:w],
                            woh[ib][:, mb * 128:(mb + 1) * 128],
                            oth[ib][sp][:, c0:c1],
                            start=(ib == 0), stop=(ib == IL // 128 - 1),
                        )
                    po = work.tile([128, SPAN], F32, tag="po", bufs=4,
                                   name="po")
                    nc.vector.tensor_scalar(
                        out=po[:, 0:w], in0=pw[:, 0:w],
                        scalar1=bias_sb[:, mb:mb + 1], scalar2=None,
                        op0=mybir.AluOpType.add,
                    )
                    nc.sync.dma_start(
                        (parts[sp] if dst is None else dst)
                        [mb * 128:(mb + 1) * 128, :], po[:, 0:w])
                return go

            def rs_store(src_parts, sp, c0, w):
                """fp32 ReduceScatter + one direct DRAM->DRAM store.

                No SBUF reload and no cast: nothing downstream of the
                collective ever occupies a compute-engine queue, so a slow
                rendezvous can only delay its own store."""
                if for_sim:
                    src = src_parts
                else:
                    rs = dram.tile([DM // 2, w], F32, name=f"rs{sp}_{c0}")
                    nc.gpsimd.collective_compute(
                        "ReduceScatter", mybir.AluOpType.add,
                        replica_groups=[[0, 1], [2, 3], [4, 5], [6, 7]],
                        ins=[src_parts.opt()], outs=[rs.opt()],
                    )
                    src = rs
                nc.sync.dma_start(
                    out_d[:, sp * SPAN + c0:sp * SPAN + c0 + w],
                    src[0:DM // 2, :])

            def rs_span(sp):
                rs_store(parts[sp], sp, 0, SPAN)

            def proj_tasks(sp, load=True):
                tasks = []
                for wt, dst in ((wqh, qth), (wkh, kth)):
                    for pb in range(IL // 128):
                        tasks.append(qk_group(wt, dst, pb, sp))
                for tb in range(4 * sp, 4 * sp + 4):
                    tasks.append(v_group(tb))
                return tasks

            def attn_s_exp(hp, qs, kb):
                """S^T matmuls + exp for key-block kb; returns the P^T tile."""
                off = kb * 128 - qs * SPAN   # <0 for off-diag
                lo = max(off, 0)             # first causal query
                sg = psum.tile([128, 2, SPAN], F32, tag="sT",
                               bufs=2, name="sg")
                for i in range(2):
                    nc.tensor.matmul(
                        sg[:, i, lo:SPAN],
                        kth[hp][kb // 4][64 * i:64 * i + 64,
                                         (kb % 4) * 128:
                                         (kb % 4) * 128 + 128],
                        qth[hp][qs][64 * i:64 * i + 64, lo:SPAN],
                        start=True, stop=True,
                    )
                pt = work.tile([128, 2, SPAN], BF16, tag="pT",
                               bufs=4, name="pt")
                nc.scalar.activation(
                    pt[:, :, lo:SPAN], sg[:, :, lo:SPAN],
                    mybir.ActivationFunctionType.Exp, scale=SCALE)
                if off >= 0:
                    # zero the strictly-upper triangle of the diagonal
                    # 128x128 block for both heads at once
                    nc.vector.tensor_tensor(
                        out=pt[:, :, lo:lo + 128],
                        in0=pt[:, :, lo:lo + 128],
                        in1=tri01[:].broadcast_to([128, 2, 128]),
                        op=mybir.AluOpType.mult,
                    )
                return pt, lo

            def attn_pv(hp, qs, kb, pt, lo, o_ps, nkb):
                for i in range(2):
                    nc.tensor.matmul(
                        o_ps[i][:, lo:SPAN],
                        vth[kb][:, 2 * hp + i, :],
                        pt[:, i, lo:SPAN],
                        start=(kb == 0), stop=(kb == nkb - 1),
                    )

            def attn_flush(hp, qs, i, o_ps, den4):
                """Copy unnormalized O^T (bf16) + its denom row out of PSUM.

                Copies run on ACT so the PSUM bank's release never queues
                behind DVE's reciprocal bursts. SBUF APs may only start at
                partition 0/32/64/96, so denominators are collected at those
                four offsets, 4 heads per collector tile."""
                h = 2 * hp + i
                t, k = h // 4, h % 4
                nc.vector.tensor_copy(den4[t][32 * k:32 * k + 1, :],
                                      o_ps[i][DH:DH + 1, :])
                nc.vector.tensor_copy(
                    oth[hp][qs][64 * i:64 * i + 64, :], o_ps[i][0:DH, :])

            def emit_half_norm(qs, den4, t, pairs):
                """Reciprocal one den4 collector (or a 33-partition slice of
                it), round it to f32r, and return per-head normalization
                finishers.

                The finisher broadcasts 1/denom to all partitions with a
                ones-column f32r matmul on the PE that reads the rounded
                collector DIRECTLY (stationary and moving share their base
                partition), then scales O^T in place on DVE. Nothing in the
                chain touches GpSimd, whose queue is blocked for the whole
                rendezvous+transfer of any in-flight ReduceScatter (measured
                15-75us), and the PE-side matmul only depends on the
                reciprocal, whose latency is hidden by deferring the
                finishers a few attention steps."""
                if len(pairs) == 2:
                    sl = slice(0, 97)
                else:
                    sl = slice(0, 33) if pairs[0] % 2 == 0 else slice(64, 97)
                nc.vector.reciprocal(den4[t][sl, :], den4[t][sl, :])
                d0s = {}
                for hp in pairs:
                    for i in range(2):
                        h = 2 * hp + i
                        k = h % 4
                        d0 = work.tile([1, SPAN], F32R, tag="d0r",
                                       bufs=6, name="d0r")
                        nc.vector.tensor_copy(
                            d0[:], den4[t][32 * k:32 * k + 1, :])
                        d0s[h] = d0

                def finisher(hp, i):
                    d0 = d0s[2 * hp + i]

                    def go():
                        rbf = psum.tile([128, SPAN], F32, tag="oT",
                                        bufs=2, name="rbf")
                        nc.tensor.matmul(rbf[:], ones_sb[:], d0[:],
                                         start=True, stop=True)
                        nc.vector.tensor_tensor(
                            out=oth[hp][qs][64 * i:64 * i + 64, :],
                            in0=oth[hp][qs][64 * i:64 * i + 64, :],
                            in1=rbf[64 * i:64 * i + 64, :],
                            op=mybir.AluOpType.mult,
                        )
                    return go
                return [finisher(hp, i) for hp in pairs for i in range(2)]

            # prologue: projections for span 0 (x span 0 already loading)
            for t in proj_tasks(0, load=False):
                t()

            normq = []       # deferred normalization finishers (cross-span)
            for sp in range(NSP):
                qs = sp
                nkb = 4 * qs + 4
                # independent PE work to weave into attention stalls:
                # next span's projections + previous span's out-proj
                pending = []
                if sp + 1 < NSP:
                    pending += [("proj", t) for t in proj_tasks(sp + 1)]
                if sp >= 1:
                    pending += [("wo", wo_group(mb, sp - 1))
                                for mb in range(DM // 128)]
                    # nothing but collectives lives on the GpSimd queue, so
                    # rs(sp-1) can ride right behind its wo chains
                    pending += [("rs", lambda sp=sp: rs_span(sp - 1))]
                nsteps = nkb * (HL // 2)
                stride = max(1, nsteps // max(1, len(pending)))
                step = 0
                den4 = [work.tile([97, SPAN], F32, tag=f"den4_{t}",
                                  bufs=2, name=f"den4_{t}")
                        for t in range(2)]
                for t in range(2):
                    nc.vector.memset(den4[t][:], 1.0)
                for hp in range(HL // 2):
                    o_ps = [psum.tile([DH + 1, SPAN], F32, tag="oT",
                                      bufs=2, name=f"o_ps{i}")
                            for i in range(2)]
                    prev = None      # (kb, pt, lo) of the un-issued P@V
                    for kb in range(nkb):
                        pt, lo = attn_s_exp(hp, qs, kb)
                        if prev is not None:
                            attn_pv(hp, qs, prev[0], prev[1], prev[2],
                                    o_ps, nkb)
                        prev = (kb, pt, lo)
                        step += 1
                        if normq and kb >= min(5, nkb - 2):
                            normq.pop(0)()
                        if INTERLEAVE and step % stride == 0 and pending:
                            if pending[0][0] != "proj":
                                # wo/rs read O^T: every pending finisher
                                # (in-place normalization) must come first
                                while normq:
                                    normq.pop(0)()
                            pending.pop(0)[1]()
                    attn_pv(hp, qs, prev[0], prev[1], prev[2], o_ps, nkb)
                    for i in range(2):
                        attn_flush(hp, qs, i, o_ps, den4)
                    if sp == NSP - 1 and hp >= 2:
                        # last span: per-pair reciprocals so the final
                        # normalization chain gating the epilogue's
                        # out-projection is as short as possible
                        fins = emit_half_norm(qs, den4, 1, [hp])
                        if hp == 3:
                            for f in fins:
                                f()
                        else:
                            normq += fins
                    elif hp % 2 == 1:
                        # den4[t] holds heads 4t..4t+3 = pairs 2t, 2t+1:
                        # reciprocal as soon as a half is complete; the
                        # finishers weave into the following steps
                        normq += emit_half_norm(qs, den4, hp // 2,
                                                [hp - 1, hp])
                while pending:
                    if pending[0][0] != "proj":
                        while normq:
                            normq.pop(0)()
                    pending.pop(0)[1]()
            # epilogue: rs(2), then the last span's out-projection in two
            # query-halves so the second half's matmuls overlap the first
            # half's ReduceScatter + store
            for f in normq:
                f()
            for h in range(2):
                for mb in range(DM // 128):
                    wo_group(mb, NSP - 1, dst=parts_last[h],
                             c0=h * (SPAN // 2), c1=(h + 1) * (SPAN // 2))()
                rs_store(parts_last[h], NSP - 1, h * (SPAN // 2), SPAN // 2)

    nc.compile()
    return nc


_program_cache = None


def make_in_maps(inputs):
    bf16 = ml_dtypes.bfloat16
    x = np.asarray(inputs["x"], dtype=np.float32)
    Wq = np.asarray(inputs["Wq"], dtype=np.float32).astype(bf16)
    Wkv = np.asarray(inputs["Wkv"], dtype=np.float32).astype(bf16)
    Wo = np.asarray(inputs["Wo"], dtype=np.float32).astype(bf16)
    bo = np.asarray(inputs["bo"], dtype=np.float32)
    in_maps = []
    for c in range(NCORES):
        b, g = c // 2, c % 2
        in_maps.append({
            "xT": np.ascontiguousarray(x[b].T).astype(bf16),
            "wq": np.ascontiguousarray(Wq[:, g * IL:(g + 1) * IL]),
            "wk": np.ascontiguousarray(Wkv[:, g * IL:(g + 1) * IL]),
            "wv": np.ascontiguousarray(Wkv[:, DM + g * IL:DM + (g + 1) * IL]),
            "wo": np.ascontiguousarray(Wo[g * IL:(g + 1) * IL, :]),
            "bias": (bo if g == 0 else np.zeros_like(bo)).reshape(DM, 1),
        })
    return in_maps


def kernel(x, Wq, Wkv, Wo, bo):
    global _program_cache
    if _program_cache is None:
        _program_cache = build_program()
    nc = _program_cache

    in_maps = make_in_maps(dict(x=x, Wq=Wq, Wkv=Wkv, Wo=Wo, bo=bo))
    res = run_bass_kernel_spmd(nc, in_maps, list(range(NCORES)))

    out = np.empty((B, N, DM), dtype=np.float32)
    for b in range(B):
        top = res.results[2 * b]["out"]       # dmodel rows 0:512
        bot = res.results[2 * b + 1]["out"]   # dmodel rows 512:1024
        out[b] = np.concatenate([top, bot], axis=0).T
    return out


# revision 57
# speedup vs baseline: 1.1457x; 1.0165x over previous
"""Causal multi-head attention (B=4, N=2048, D=1024, H=16, Dh=64) on 8 TRN2 cores.

Sharding: core c handles batch b=c//2 and head-group g=c%2 (8 of 16 heads).
Megatron-style: Wq/Wkv column-parallel, Wo row-parallel; the per-pair partial
outputs are combined with a bf16 ReduceScatter(add) over core pairs {2b, 2b+1},
then cast back to fp32 on-device.

Everything on-device runs in a transposed layout ([feature, token]) so that no
PE transposes are needed anywhere:
  Qt/Kt = W-stationary matmuls of xT            -> [inner, tok]
  S^T   = Kt-stationary, Qt-moving              -> [key, query]  (2 heads row-packed)
  P^T   = exp(scale*S^T) via ACT, 0/1-masked    -> [key, query]  bf16
  O^T   = V'-stationary ([V | ones]), P^T-moving-> [65, query]   (row 64 = softmax denom)
  out^T = Wo-stationary, O^T-moving             -> [dmodel, tok]
The host pre-transposes and pre-casts x / weights to bf16, so the device does
no fp32->bf16 conversion and input DMA bytes are halved.

Perf structure:
  - warm-up matmuls on scratch SBUF keep the PE HAM clock at 2.4 GHz while the
    initial DMAs stream in;
  - the P@V stage is software-pipelined one key-block behind exp so the PE
    never waits on the ACT engine;
  - softmax denominators are collected at partition offsets {0,32,64,96} (the
    only legal SBUF AP start partitions) and inverted in batched DVE
    reciprocals (4 heads per call) instead of 32 single-partition ones; the
    unnormalized O^T is flushed to SBUF as bf16 so PSUM frees immediately;
  - 1/denom is partition-broadcast with a ones-column f32r matmul on the PE
    (never GpSimd: any in-flight ReduceScatter blocks that queue for its full
    15-75us rendezvous+transfer) and the per-head finishers are woven a few
    attention steps past the reciprocal so the PE stream never waits on DVE;
  - the ReduceScatter output reload is cast to fp32 on ACT, keeping the
    CC-gated copy off the in-order DVE queue;
  - attention-output PSUM tiles rotate through 3 banks so the flush of
    head-pair k overlaps the accumulation of head-pair k+1;
  - projections / output-projection / ReduceScatter of neighbouring spans are
    woven between attention steps to fill PE gaps; the last span does
    per-pair reciprocals and a two-half out-proj / ReduceScatter / store
    pipeline to shorten the kernel tail.
"""

import sys

sys.path.insert(0, "/opt/trn_rl_repo")

import ml_dtypes
import numpy as np

import concourse.bass as bass  # noqa: F401  (kept for parity with framework)
import concourse.mybir as mybir
from concourse import bacc, tile
from concourse.bass_utils import run_bass_kernel_spmd

F32 = mybir.dt.float32
BF16 = mybir.dt.bfloat16
FP8 = mybir.dt.float8e4

B = 4
N = 2048
DM = 1024          # d_model
H = 16
DH = 64
HL = 8             # local heads per core
IL = HL * DH       # 512, local inner dim
SCALE = DH ** -0.5
SPAN = 512         # query-span / matmul moving size
NSP = N // SPAN    # 4
NKB = N // 128     # 16 key/token blocks
NCORES = 8
NWARM = 20         # PE warm-up matmuls covering the initial DMA window
INTERLEAVE = True


def build_program(for_sim=False):
    nc = bacc.Bacc("TRN2", target_bir_lowering=False, debug=False,
                   num_devices=1 if for_sim else NCORES)

    xT_d = nc.dram_tensor("xT", [DM, N], BF16, kind="ExternalInput").ap()
    wq_d = nc.dram_tensor("wq", [DM, IL], BF16, kind="ExternalInput").ap()
    wk_d = nc.dram_tensor("wk", [DM, IL], BF16, kind="ExternalInput").ap()
    wv_d = nc.dram_tensor("wv", [DM, IL], BF16, kind="ExternalInput").ap()
    wo_d = nc.dram_tensor("wo", [IL, DM], BF16, kind="ExternalInput").ap()
    bias_d = nc.dram_tensor("bias", [DM, 1], F32, kind="ExternalInput").ap()
    out_d = nc.dram_tensor("out", [DM // 2, N], F32, kind="ExternalOutput").ap()

    with tile.TileContext(nc) as tc:
        with (
            tc.tile_pool(name="weights", bufs=1) as wpool,
            tc.tile_pool(name="acts", bufs=1) as apool,
            tc.tile_pool(name="work", bufs=3) as work,
            tc.tile_pool(name="psum", bufs=1, space="PSUM") as psum,
            tc.tile_pool(name="dram", bufs=1, space="DRAM") as dram,
        ):
            # ---------- stage 0: PE warm-up + loads (no casts needed) ----
            warm_sb = wpool.tile([128, SPAN], BF16, name="warm_sb")
            nc.vector.memset(warm_sb[:], 0.0)
            ones_f = wpool.tile([1, 128], F32, name="ones_f")
            nc.vector.memset(ones_f[:], 1.0)
            ones_sb = wpool.tile([1, 128], F32R, name="ones_sb")
            nc.vector.tensor_copy(ones_sb[:], ones_f[:])
            for _ in range(NWARM):
                wp = psum.tile([128, SPAN], F32, tag="projrb", bufs=2,
                               name="warm_ps")
                nc.tensor.matmul(wp[:], warm_sb[:, 0:128], warm_sb[:],
                                 start=True, stop=True)

            xh = [apool.tile([128, N], BF16, name=f"xh{pb}", tag=f"xh{pb}")
                  for pb in range(DM // 128)]

            def load_x_span(sp):
                for pb in range(DM // 128):
                    nc.sync.dma_start(
                        xh[pb][:, sp * SPAN:(sp + 1) * SPAN],
                        xT_d[pb * 128:(pb + 1) * 128,
                             sp * SPAN:(sp + 1) * SPAN])

            def load_w(src, n_pb, ncols, nm):
                tiles = []
                for pb in range(n_pb):
                    t = wpool.tile([128, ncols], BF16, name=f"{nm}{pb}",
                                   tag=f"{nm}{pb}")
                    nc.sync.dma_start(t[:], src[pb * 128:(pb + 1) * 128, :])
                    tiles.append(t)
                return tiles

            load_x_span(0)                      # span-0 x first: unblocks PE
            wqh = load_w(wq_d, DM // 128, IL, "wqh")
            wkh = load_w(wk_d, DM // 128, IL, "wkh")
            wvh = load_w(wv_d, DM // 128, IL, "wvh")
            for sp in range(1, NSP):            # prefetch the rest of x
                load_x_span(sp)
            woh = load_w(wo_d, IL // 128, DM, "woh")

            bias_sb = wpool.tile([128, DM // 128], F32, name="bias_sb")
            for mb in range(DM // 128):
                nc.sync.dma_start(bias_sb[:, mb:mb + 1],
                                  bias_d[mb * 128:(mb + 1) * 128, :])

            # 0/1 lower-triangle mask (keep query >= key within a diag block)
            tri_f = work.tile([128, 128], F32, tag="tri_f", bufs=1)
            nc.gpsimd.memset(tri_f[:], 1.0)
            nc.gpsimd.affine_select(
                out=tri_f[:], in_=tri_f[:],
                compare_op=mybir.AluOpType.is_ge,
                fill=0.0, base=0, channel_multiplier=-1,
                pattern=[[1, 128]],
            )
            tri01 = wpool.tile([128, 1, 128], BF16, name="tri01")
            nc.vector.tensor_copy(tri01[:, 0, :], tri_f[:])

            # per-span activation tiles
            qth = [[apool.tile([128, SPAN], BF16, name=f"qt{pb}_{sp}",
                               tag=f"qt{pb}_{sp}")
                    for sp in range(NSP)] for pb in range(IL // 128)]
            kth = [[apool.tile([128, SPAN], BF16, name=f"kt{pb}_{sp}",
                               tag=f"kt{pb}_{sp}")
                    for sp in range(NSP)] for pb in range(IL // 128)]
            vth = [apool.tile([128, HL, DH + 1], BF16, name=f"vt{tb}",
                              tag=f"vt{tb}") for tb in range(NKB)]
            oth = [[apool.tile([128, SPAN], BF16, name=f"ot{pb}_{sp}",
                               tag=f"ot{pb}_{sp}")
                    for sp in range(NSP)] for pb in range(IL // 128)]
            # last span's partials are split into two query-halves so its
            # out-proj / ReduceScatter / store pipeline has a shorter tail
            parts = [dram.tile([DM, SPAN], F32, name=f"part{sp}")
                     for sp in range(NSP)]

            def qk_group(wt, dst, pb, sp):
                def go():
                    pp = psum.tile([128, SPAN], F32, tag="projrb",
                                   bufs=2, name="pp")
                    for kk in range(DM // 128):
                        nc.tensor.matmul(
                            pp[:],
                            wt[kk][:, pb * 128:(pb + 1) * 128],
                            xh[kk][:, sp * SPAN:(sp + 1) * SPAN],
                            start=(kk == 0), stop=(kk == DM // 128 - 1),
                        )
                    # copy-out on ACT: frees the PSUM slot without queuing
                    # behind DVE's span-boundary normalization bursts
                    nc.scalar.copy(dst[pb][sp][:], pp[:])
                return go

            def v_group(tb):
                def go():
                    pp = psum.tile([128, IL], F32, tag="projrb", bufs=2,
                                   name="ppv")
                    for kk in range(DM // 128):
                        nc.tensor.matmul(
                            pp[:], xh[kk][:, tb * 128:(tb + 1) * 128],
                            wvh[kk][:],
                            start=(kk == 0), stop=(kk == DM // 128 - 1),
                        )
                    nc.scalar.copy(
                        vth[tb][:, :, 0:DH],
                        pp[:].rearrange("p (h d) -> p h d", h=HL))
                    nc.vector.memset(vth[tb][:, :, DH:DH + 1], 1.0)
                return go

            def wo_group(mb, sp, dst=None, c0=0, c1=SPAN):
                w = c1 - c0

                def go():
                    pw = psum.tile([128, SPAN], F32, tag="projrb", bufs=2,
                                   name="pw")
                    for ib in range(IL // 128):
                        nc.tensor.matmul(
                            pw[:, 0:w],
                            woh[ib][:, mb * 128:(mb + 1) * 128],
                            oth[ib][sp][:, c0:c1],
                            start=(ib == 0), stop=(ib == IL // 128 - 1),
                        )
                    po = work.tile([128, SPAN], F32, tag="po", bufs=4,
                                   name="po")
                    nc.vector.tensor_scalar(
                        out=po[:, 0:w], in0=pw[:, 0:w],
                        scalar1=bias_sb[:, mb:mb + 1], scalar2=None,
                        op0=mybir.AluOpType.add,
                    )
                    nc.sync.dma_start(
                        (parts[sp] if dst is None else dst)
                        [mb * 128:(mb + 1) * 128, :], po[:, 0:w])
                return go

            def rs_store(src_parts, sp, c0, w):
                """fp32 ReduceScatter + one direct DRAM->DRAM store.

                No SBUF reload and no cast: nothing downstream of the
                collective ever occupies a compute-engine queue, so a slow
                rendezvous can only delay its own store."""
                if for_sim:
                    src = src_parts
                else:
                    rs = dram.tile([DM // 2, w], F32, name=f"rs{sp}_{c0}")
                    nc.gpsimd.collective_compute(
                        "ReduceScatter", mybir.AluOpType.add,
                        replica_groups=[[0, 1], [2, 3], [4, 5], [6, 7]],
                        ins=[src_parts.opt()], outs=[rs.opt()],
                    )
                    src = rs
                nc.sync.dma_start(
                    out_d[:, sp * SPAN + c0:sp * SPAN + c0 + w],
                    src[0:DM // 2, :])

            def rs_span(sp):
                rs_store(parts[sp], sp, 0, SPAN)

            def proj_tasks(sp, load=True):
                tasks = []
                for wt, dst in ((wqh, qth), (wkh, kth)):
                    for pb in range(IL // 128):
                        tasks.append(qk_group(wt, dst, pb, sp))
                for tb in range(4 * sp, 4 * sp + 4):
                    tasks.append(v_group(tb))
                return tasks

            def attn_s_exp(hp, qs, kb):
                """S^T matmuls + exp for key-block kb; returns the P^T tile."""
                off = kb * 128 - qs * SPAN   # <0 for off-diag
                lo = max(off, 0)             # first causal query
                sg = psum.tile([128, 2, SPAN], F32, tag="sT",
                               bufs=2, name="sg")
                for i in range(2):
                    nc.tensor.matmul(
                        sg[:, i, lo:SPAN],
                        kth[hp][kb // 4][64 * i:64 * i + 64,
                                         (kb % 4) * 128:
                                         (kb % 4) * 128 + 128],
                        qth[hp][qs][64 * i:64 * i + 64, lo:SPAN],
                        start=True, stop=True,
                    )
                pt = work.tile([128, 2, SPAN], BF16, tag="pT",
                               bufs=4, name="pt")
                nc.scalar.activation(
                    pt[:, :, lo:SPAN], sg[:, :, lo:SPAN],
                    mybir.ActivationFunctionType.Exp, scale=SCALE)
                if off >= 0:
                    # zero the strictly-upper triangle of the diagonal
                    # 128x128 block for both heads at once
                    nc.vector.tensor_tensor(
                        out=pt[:, :, lo:lo + 128],
                        in0=pt[:, :, lo:lo + 128],
                        in1=tri01[:].broadcast_to([128, 2, 128]),
                        op=mybir.AluOpType.mult,
                    )
                return pt, lo

            def attn_pv(hp, qs, kb, pt, lo, o_ps, nkb):
                for i in range(2):
                    nc.tensor.matmul(
                        o_ps[i][:, lo:SPAN],
                        vth[kb][:, 2 * hp + i, :],
                        pt[:, i, lo:SPAN],
                        start=(kb == 0), stop=(kb == nkb - 1),
                    )

            def attn_flush(hp, qs, i, o_ps, den4):
                """Copy unnormalized O^T (bf16) + its denom row out of PSUM.

                Copies run on ACT so the PSUM bank's release never queues
                behind DVE's reciprocal bursts. SBUF APs may only start at
                partition 0/32/64/96, so denominators are collected at those
                four offsets, 4 heads per collector tile."""
                h = 2 * hp + i
                t, k = h // 4, h % 4
                nc.vector.tensor_copy(den4[t][32 * k:32 * k + 1, :],
                                      o_ps[i][DH:DH + 1, :])
                nc.vector.tensor_copy(
                    oth[hp][qs][64 * i:64 * i + 64, :], o_ps[i][0:DH, :])

            def emit_half_norm(qs, den4, t, pairs):
                """Reciprocal one den4 collector (or a 33-partition slice of
                it), round it to f32r, and return per-head normalization
                finishers.

                The finisher broadcasts 1/denom to all partitions with a
                ones-column f32r matmul on the PE that reads the rounded
                collector DIRECTLY (stationary and moving share their base
                partition), then scales O^T in place on DVE. Nothing in the
                chain touches GpSimd, whose queue is blocked for the whole
                rendezvous+transfer of any in-flight ReduceScatter (measured
                15-75us), and the PE-side matmul only depends on the
                reciprocal, whose latency is hidden by deferring the
                finishers a few attention steps."""
                if len(pairs) == 2:
                    sl = slice(0, 97)
                else:
                    sl = slice(0, 33) if pairs[0] % 2 == 0 else slice(64, 97)
                nc.vector.reciprocal(den4[t][sl, :], den4[t][sl, :])
                d0s = {}
                for hp in pairs:
                    for i in range(2):
                        h = 2 * hp + i
                        k = h % 4
                        d0 = work.tile([1, SPAN], F32R, tag="d0r",
                                       bufs=6, name="d0r")
                        nc.vector.tensor_copy(
                            d0[:], den4[t][32 * k:32 * k + 1, :])
                        d0s[h] = d0

                def finisher(hp, i):
                    d0 = d0s[2 * hp + i]

                    def go():
                        rbf = psum.tile([128, SPAN], F32, tag="oT",
                                        bufs=2, name="rbf")
                        nc.tensor.matmul(rbf[:], ones_sb[:], d0[:],
                                         start=True, stop=True)
                        nc.vector.tensor_tensor(
                            out=oth[hp][qs][64 * i:64 * i + 64, :],
                            in0=oth[hp][qs][64 * i:64 * i + 64, :],
                            in1=rbf[64 * i:64 * i + 64, :],
                            op=mybir.AluOpType.mult,
                        )
                    return go
                return [finisher(hp, i) for hp in pairs for i in range(2)]

            # prologue: projections for span 0 (x span 0 already loading)
            for t in proj_tasks(0, load=False):
                t()

            normq = []       # deferred normalization finishers (cross-span)
            for sp in range(NSP):
                qs = sp
                nkb = 4 * qs + 4
                # independent PE work to weave into attention stalls:
                # next span's projections + previous span's out-proj
                pending = []
                if sp + 1 < NSP:
                    pending += [("proj", t) for t in proj_tasks(sp + 1)]
                if sp >= 1:
                    pending += [("wo", wo_group(mb, sp - 1))
                                for mb in range(DM // 128)]
                    # nothing but collectives lives on the GpSimd queue, so
                    # rs(sp-1) can ride right behind its wo chains
                    pending += [("rs", lambda sp=sp: rs_span(sp - 1))]
                nsteps = nkb * (HL // 2)
                stride = max(1, nsteps // max(1, len(pending)))
                step = 0
                den4 = [work.tile([97, SPAN], F32, tag=f"den4_{t}",
                                  bufs=2, name=f"den4_{t}")
                        for t in range(2)]
                for t in range(2):
                    nc.vector.memset(den4[t][:], 1.0)
                for hp in range(HL // 2):
                    o_ps = [psum.tile([DH + 1, SPAN], F32, tag="oT",
                                      bufs=2, name=f"o_ps{i}")
                            for i in range(2)]
                    prev = None      # (kb, pt, lo) of the un-issued P@V
                    for kb in range(nkb):
                        pt, lo = attn_s_exp(hp, qs, kb)
                        if prev is not None:
                            attn_pv(hp, qs, prev[0], prev[1], prev[2],
                                    o_ps, nkb)
                        prev = (kb, pt, lo)
                        step += 1
                        if normq and kb >= min(7, nkb - 2):
                            normq.pop(0)()
                        if INTERLEAVE and step % stride == 0 and pending:
                            if pending[0][0] != "proj":
                                # wo/rs read O^T: every pending finisher
                                # (in-place normalization) must come first
                                while normq:
                                    normq.pop(0)()
                            pending.pop(0)[1]()
                    attn_pv(hp, qs, prev[0], prev[1], prev[2], o_ps, nkb)
                    for i in range(2):
                        attn_flush(hp, qs, i, o_ps, den4)
                    if sp == NSP - 1 and hp >= 2:
                        # last span: per-pair reciprocals so the final
                        # normalization chain gating the epilogue's
                        # out-projection is as short as possible
                        fins = emit_half_norm(qs, den4, 1, [hp])
                        if hp == 3:
                            for f in fins:
                                f()
                        else:
                            normq += fins
                    elif hp % 2 == 1:
                        # den4[t] holds heads 4t..4t+3 = pairs 2t, 2t+1:
                        # reciprocal as soon as a half is complete; the
                        # finishers weave into the following steps
                        normq += emit_half_norm(qs, den4, hp // 2,
                                                [hp - 1, hp])
                while pending:
                    if pending[0][0] != "proj":
                        while normq:
                            normq.pop(0)()
                    pending.pop(0)[1]()
            # epilogue: the last span's out-projection, then one
            # ReduceScatter + direct store (a single collective has a lower
            # total latency floor than two serialized half-span ones)
            for f in normq:
                f()
            for mb in range(DM // 128):
                wo_group(mb, NSP - 1)()
            rs_span(NSP - 1)

    nc.compile()
    return nc


_program_cache = None


def make_in_maps(inputs):
    bf16 = ml_dtypes.bfloat16
    x = np.asarray(inputs["x"], dtype=np.float32)
    Wq = np.asarray(inputs["Wq"], dtype=np.float32).astype(bf16)
    Wkv = np.asarray(inputs["Wkv"], dtype=np.float32).astype(bf16)
    Wo = np.asarray(inputs["Wo"], dtype=np.float32).astype(bf16)
    bo = np.asarray(inputs["bo"], dtype=np.float32)
    in_maps = []
    for c in range(NCORES):
        b, g = c // 2, c % 2
        in_maps.append({
            "xT": np.ascontiguousarray(x[b].T).astype(bf16),
            "wq": np.ascontiguousarray(Wq[:, g * IL:(g + 1) * IL]),
            "wk": np.ascontiguousarray(Wkv[:, g * IL:(g + 1) * IL]),
            "wv": np.ascontiguousarray(Wkv[:, DM + g * IL:DM + (g + 1) * IL]),
            "wo": np.ascontiguousarray(Wo[g * IL:(g + 1) * IL, :]),
            "bias": (bo if g == 0 else np.zeros_like(bo)).reshape(DM, 1),
        })
    return in_maps


def kernel(x, Wq, Wkv, Wo, bo):
    global _program_cache
    if _program_cache is None:
        _program_cache = build_program()
    nc = _program_cache

    in_maps = make_in_maps(dict(x=x, Wq=Wq, Wkv=Wkv, Wo=Wo, bo=bo))
    res = run_bass_kernel_spmd(nc, in_maps, list(range(NCORES)))

    out = np.empty((B, N, DM), dtype=np.float32)
    for b in range(B):
        top = res.results[2 * b]["out"]       # dmodel rows 0:512
        bot = res.results[2 * b + 1]["out"]   # dmodel rows 512:1024
        out[b] = np.concatenate([top, bot], axis=0).T
    return out
